# revision 7
# baseline (speedup 1.0000x reference)
"""AttentionFlowLayer (BiDAF-style) Trainium2 kernel.

Full inputs in, full output out. Data-parallel over batch B=32 across 8
NeuronCores (4 batches per core, no cross-core communication).

Math (per batch b):
    S[i,j]  = main[i,j] + hw[i] + uw[j] + b,  main = (h * w_hu) @ u^T
    a[i,j]  = softmax_j(where(u_mask, S, NEG))      -> hw[i], b cancel
    b_t[i,j]= softmax_i(where(h_mask, S, NEG))      -> uw[j], b cancel
    U~ = a @ u ; H~ = b_t @ (a^T @ h)               (avoids [Lh,Lh] interm.)
    out = [h, U~, h*U~, h*H~]

Device-side decomposition (unnormalized-softmax algebra, no max pass —
exponents are O(10), far inside f32 range):
    E[i,j]  = exp(main + uwm[j])        uwm = uw + (u_mask ? 0 : NEG)
    s[i]    = sum_j E ; r = 1/s ; a = E * r
    eb[i]   = h_mask ? exp(hw[i]) : 0   (host-folded)
    ebs     = eb * s
    Z[j]    = sum_i a[i,j] * ebs[i]     (= b_t denominator, rescaled)
    G       = a^T @ h ; G' = G / (Z + tiny)
    H~[i,:] = ebs[i] * (a @ G')[i,:]
"""

import sys

if "/opt/trn_rl_repo" not in sys.path:
    sys.path.insert(0, "/opt/trn_rl_repo")

import numpy as np
from contextlib import ExitStack

import concourse.bass as bass
import concourse.bacc as bacc
import concourse.tile as tile
from concourse import mybir
from concourse.bass_utils import run_bass_kernel_spmd
from concourse.masks import make_identity

B, LH, LU, H = 32, 1024, 128, 256
NCORES = 8
BP = B // NCORES          # batches per core
NT = LH // 128            # 8 i-tiles of 128 rows
NEG = -1e30

F32 = mybir.dt.float32
F32R = mybir.dt.float32r
ts = bass.ts


def _body(tc):
    nc = tc.nc
    h_ext = nc.declare_dram_parameter("h", [BP, LH, H], F32, isOutput=False)
    hT_ext = nc.declare_dram_parameter("hT", [BP, H, LH], F32, isOutput=False)
    u_ext = nc.declare_dram_parameter("u", [BP, LU, H], F32, isOutput=False)
    uTw_ext = nc.declare_dram_parameter("uTw", [BP, H, LU], F32, isOutput=False)
    eb_ext = nc.declare_dram_parameter("eb", [BP, LH], F32, isOutput=False)
    uwm_ext = nc.declare_dram_parameter("uwm", [BP, LU], F32, isOutput=False)
    out_ext = nc.declare_dram_parameter("out", [BP, LH, 4 * H], F32, isOutput=True)

    with ExitStack() as ctx:
        const = ctx.enter_context(tc.tile_pool(name="const", bufs=1))
        p_h = ctx.enter_context(tc.tile_pool(name="p_h", bufs=2))
        p_hT = ctx.enter_context(tc.tile_pool(name="p_hT", bufs=2))
        p_u = ctx.enter_context(tc.tile_pool(name="p_u", bufs=2))
        p_E = ctx.enter_context(tc.tile_pool(name="p_E", bufs=2))
        p_a = ctx.enter_context(tc.tile_pool(name="p_a", bufs=2))
        p_aT = ctx.enter_context(tc.tile_pool(name="p_aT", bufs=2))
        p_G = ctx.enter_context(tc.tile_pool(name="p_G", bufs=2))
        p_small = ctx.enter_context(tc.tile_pool(name="p_small", bufs=4))
        p_out = ctx.enter_context(tc.tile_pool(name="p_out", bufs=2 * NT + 2))
        ps_S = ctx.enter_context(tc.tile_pool(name="ps_S", bufs=1, space="PSUM"))
        ps_T = ctx.enter_context(tc.tile_pool(name="ps_T", bufs=2, space="PSUM"))
        ps_mm = ctx.enter_context(tc.tile_pool(name="ps_mm", bufs=2, space="PSUM"))
        ps_G = ctx.enter_context(tc.tile_pool(name="ps_G", bufs=1, space="PSUM"))
        ps_Z = ctx.enter_context(tc.tile_pool(name="ps_Z", bufs=1, space="PSUM"))

        ident = const.tile([128, 128], F32)
        make_identity(nc, ident)

        state = {}

        def stage1(bb):
            h_sb = p_h.tile([128, NT, H], F32)
            nc.sync.dma_start(
                out=h_sb, in_=h_ext[bb].rearrange("(t p) c -> p t c", p=128)
            )
            hT_sb = p_hT.tile([128, 2, LH], F32)
            nc.sync.dma_start(
                out=hT_sb, in_=hT_ext[bb].rearrange("(k p) i -> p k i", p=128)
            )
            u_sb = p_u.tile([128, H], F32)
            nc.sync.dma_start(out=u_sb, in_=u_ext[bb])
            uTw_sb = p_u.tile([128, 2, LU], F32)
            nc.sync.dma_start(
                out=uTw_sb, in_=uTw_ext[bb].rearrange("(k p) j -> p k j", p=128)
            )
            eb_sb = p_small.tile([128, NT], F32)
            nc.sync.dma_start(
                out=eb_sb, in_=eb_ext[bb].rearrange("(t p) -> p t", p=128)
            )
            # uwm row broadcast to all 128 partitions via DMA (step-0 AP).
            uwm_bc = p_small.tile([128, LU], F32)
            src = uwm_ext[bb]
            nc.sync.dma_start(
                out=uwm_bc,
                in_=bass.AP(tensor=src.tensor, offset=src.offset,
                            ap=[[0, 128]] + list(src.ap)),
            )

            # S_main[i-tile t, j] accumulated in PSUM over the two c-chunks.
            s_psum = ps_S.tile([128, NT, LU], F32)
            for t in range(NT):
                for k in range(2):
                    nc.tensor.matmul(
                        s_psum[:, t, :],
                        hT_sb[:, k, ts(t, 128)],
                        uTw_sb[:, k, :],
                        start=(k == 0),
                        stop=(k == 1),
                    )

            # E = exp(S_main + uwm[j]): DVE adds the row (broadcast over t),
            # ACT exponentiates in place.
            E_all = p_E.tile([128, NT, LU], F32)
            uap = uwm_bc[:, :]
            uwm_3d = bass.AP(tensor=uap.tensor, offset=uap.offset,
                             ap=[list(uap.ap[0]), [0, NT], list(uap.ap[1])])
            nc.vector.tensor_add(E_all, s_psum, uwm_3d)
            nc.scalar.activation(E_all, E_all, mybir.ActivationFunctionType.Exp)
            ssum = p_small.tile([128, NT], F32)
            nc.vector.reduce_sum(ssum, E_all, axis=mybir.AxisListType.X)
            r = p_small.tile([128, NT], F32)
            nc.vector.reciprocal(r, ssum)
            a_all = p_a.tile([128, NT, LU], F32)
            nc.vector.tensor_mul(a_all, E_all, r.broadcast_to((128, NT, LU)))
            ebs = p_small.tile([128, NT], F32)
            nc.vector.tensor_mul(ebs, eb_sb, ssum)

            # a^T per i-tile via PE transpose (4 tiles per PSUM bank).
            aT_all = p_aT.tile([128, NT, 128], F32)
            for g in range(2):
                tp = ps_T.tile([128, 4, 128], F32)
                for q in range(4):
                    nc.tensor.transpose(tp[:, q, :], a_all[:, g * 4 + q, :], ident)
                nc.scalar.copy(aT_all[:, g * 4 : g * 4 + 4, :], tp)

            # U~ per tile -> straight into the output staging tile cols 0:H.
            o_tiles = []
            for t in range(NT):
                o_sb = p_out.tile([128, 3 * H], F32)
                up = ps_mm.tile([128, H], F32, tag="mm")
                nc.tensor.matmul(
                    up,
                    aT_all[:, t, :],
                    u_sb,
                )
                nc.scalar.copy(o_sb[:, 0:H], up)
                o_tiles.append(o_sb)

            # G = a^T @ h and Z = a^T @ ebs, both accumulated over i-tiles.
            g_psum = ps_G.tile([128, H], F32)
            for t in range(NT):
                nc.tensor.matmul(
                    g_psum,
                    a_all[:, t, :],
                    h_sb[:, t, :],
                    start=(t == 0),
                    stop=(t == NT - 1),
                )
            z_psum = ps_Z.tile([128, 1], F32)
            for t in range(NT):
                nc.tensor.matmul(
                    z_psum,
                    a_all[:, t, :],
                    ebs[:, t : t + 1],
                    start=(t == 0),
                    stop=(t == NT - 1),
                )
            G_sb = p_G.tile([128, H], F32)
            nc.scalar.copy(G_sb, g_psum)
            Z_sb = p_small.tile([128, 1], F32)
            nc.scalar.copy(Z_sb, z_psum)

            state[bb] = (h_sb, aT_all, G_sb, Z_sb, ebs, o_tiles)

        def stage2(bb):
            h_sb, aT_all, G_sb, Z_sb, ebs, o_tiles = state.pop(bb)
            rz = p_small.tile([128, 1], F32)
            nc.vector.tensor_scalar_add(rz, Z_sb, 1e-30)
            nc.vector.reciprocal(rz, rz)
            Gp = p_G.tile([128, H], F32)
            nc.vector.tensor_scalar_mul(Gp, G_sb, rz)

            for t in range(NT):
                o_sb = o_tiles[t]
                ah = ps_mm.tile([128, H], F32, tag="mm")
                nc.tensor.matmul(
                    ah,
                    aT_all[:, t, :],
                    Gp,
                )
                # cols H:2H = h * U~, cols 2H:3H = ebs * h * (a@G')
                nc.gpsimd.tensor_mul(o_sb[:, H : 2 * H], h_sb[:, t, :], o_sb[:, 0:H])
                nc.vector.tensor_mul(o_sb[:, 2 * H : 3 * H], h_sb[:, t, :], ah)
                nc.vector.tensor_scalar_mul(
                    o_sb[:, 2 * H : 3 * H],
                    o_sb[:, 2 * H : 3 * H],
                    ebs[:, t : t + 1],
                )
                nc.sync.dma_start(out=out_ext[bb, ts(t, 128), 0:H], in_=h_sb[:, t, :])
                nc.sync.dma_start(out=out_ext[bb, ts(t, 128), H : 4 * H], in_=o_sb)

        for bb in range(BP):
            stage1(bb)
            if bb >= 1:
                stage2(bb - 1)
        stage2(BP - 1)


_NC_CACHE = None


def _build_nc():
    global _NC_CACHE
    if _NC_CACHE is None:
        nc = bacc.Bacc("TRN2", target_bir_lowering=False, enable_partition_id=False)
        with tile.TileContext(nc) as tc:
            _body(tc)
        nc.finalize()
        _NC_CACHE = nc
    return _NC_CACHE


def _make_in_maps(h, u, h_mask, u_mask, w, b):
    h = np.ascontiguousarray(h, dtype=np.float32)
    u = np.ascontiguousarray(u, dtype=np.float32)
    w = np.asarray(w, dtype=np.float32)
    w_h, w_u, w_hu = w[:H], w[H : 2 * H], w[2 * H :]
    hT = np.ascontiguousarray(h.transpose(0, 2, 1))
    uTw = np.ascontiguousarray((u * w_hu).transpose(0, 2, 1))
    eb = np.where(h_mask, np.exp(h @ w_h), np.float32(0.0)).astype(np.float32)
    uwm = (u @ w_u + np.where(u_mask, np.float32(0.0), np.float32(NEG))).astype(
        np.float32
    )
    in_maps = []
    for i in range(NCORES):
        s = slice(i * BP, (i + 1) * BP)
        in_maps.append(
            {
                "h": h[s],
                "hT": hT[s],
                "u": u[s],
                "uTw": uTw[s],
                "eb": eb[s],
                "uwm": uwm[s],
            }
        )
    return in_maps


def kernel(h, u, h_mask, u_mask, w, b):
    nc = _build_nc()
    in_maps = _make_in_maps(h, u, h_mask, u_mask, w, b)
    res = run_bass_kernel_spmd(nc, in_maps, core_ids=list(range(NCORES)))
    return np.concatenate([res.results[i]["out"] for i in range(NCORES)], axis=0)
